# revision 1
# baseline (speedup 1.0000x reference)
"""Trainium2 Bass kernel for nn_DecoderModule_16853451669850 (8 NeuronCores).

Decoder block: x + MHA(x) -> LN -> +FFN -> LN.
Sharding: heads 2c,2c+1 on core c for attention (tensor-parallel over heads);
tokens [256c, 256c+256) on core c for pool+LN+FFN (sequence-parallel).
A single 8-way AllToAll (bf16) pivots between the two shardings.

Precision: float32r (e8m11) for the x/W_Q/W_K/Q/K/score path (the causal
softmax here is near-one-hot with huge logits, so bf16 flips argmaxes);
bf16 for V/P/pool/FFN operands; fp32 PSUM accumulation, softmax statistics,
layernorms and residuals.
"""

import numpy as np
import ml_dtypes
import concourse.bacc as bacc
import concourse.mybir as mybir
import concourse.tile as tile
from concourse.alu_op_type import AluOpType

F32, F32R, BF16 = mybir.dt.float32, mybir.dt.float32r, mybir.dt.bfloat16
AF = mybir.ActivationFunctionType
H, D, E, N, F = 16, 128, 2048, 2048, 8192
NCORE = 8
NB_TOK = N // 128          # 16 token blocks of 128 (per head attention)
TOKPC = N // NCORE         # 256 tokens per core
SCALE = 1.0 / np.sqrt(np.float32(D))
EPS = 1e-5
NEG = -1.0e30


def build_nc(dbg=()):
    nc = bacc.Bacc("TRN2", target_bir_lowering=False, debug=False)
    dt_in = {}

    def param(name, shape, dtype):
        dt_in[name] = dtype
        return nc.declare_dram_parameter(name, list(shape), dtype, isOutput=False)

    xT = param("xT", [E, N], F32R)            # x transposed, f32r-rounded
    wq = param("wq", [E, 2 * D], F32R)        # this core's 2 heads
    wk = param("wk", [E, 2 * D], F32R)
    wv = param("wv", [E, 2 * D], F32R)
    wpool = param("wpool", [H * D, E], BF16)
    w1 = param("w1", [E, F], BF16)
    w2 = param("w2", [F, E], BF16)
    b1bc = param("b1bc", [128, F], BF16)      # b1 broadcast along partitions
    b2bc = param("b2bc", [128, E], F32)
    beta_bc = param("beta_bc", [128, E], F32)
    gcol = param("gcol", [128, 1], F32)
    xblk = param("xblk", [TOKPC, E], F32)     # this core's token rows of x
    maskc = param("maskc", [128, 128], F32)   # 0 if m<=n else -1e30 (n=part)
    maskt = param("maskt", [128, 128], F32)   # 0 if m<=n else -1e30 (m=part)
    ones1 = param("ones1", [1, 128], F32)
    epscol = param("epscol", [128, 1], F32)
    ident = param("ident", [128, 128], F32)

    out_blk = nc.declare_dram_parameter("out_blk", [TOKPC, E], F32, isOutput=True)
    dbg_outs = {}
    def dbg_param(name, shape, dtype=F32):
        if name in dbg:
            dbg_outs[name] = nc.declare_dram_parameter(name, list(shape), dtype, isOutput=True)
        return dbg_outs.get(name)

    d_qt = dbg_param("d_qt", [128, N])
    d_kt = dbg_param("d_kt", [128, N])
    d_vt = dbg_param("d_vt", [128, N])
    d_v = dbg_param("d_v", [128, 16 * 128], BF16)
    d_heads = dbg_param("d_heads", [128, N], BF16)
    d_stat = dbg_param("d_stat", [1, N])
    d_pool = dbg_param("d_pool", [TOKPC, E])
    d_y = dbg_param("d_y", [TOKPC, E])
    d_h = dbg_param("d_h", [TOKPC, F], BF16)

    with tile.TileContext(nc) as tc:
        _emit(nc, tc, locals())
    nc.compile()
    return nc, dt_in


def _emit(nc, tc, t):
    xT, wq, wk, wv = t["xT"], t["wq"], t["wk"], t["wv"]
    wpool, w1, w2 = t["wpool"], t["w1"], t["w2"]
    b1bc, b2bc, beta_bc, gcol = t["b1bc"], t["b2bc"], t["beta_bc"], t["gcol"]
    xblk, maskc, maskt, ones1, ident = t["xblk"], t["maskc"], t["maskt"], t["ones1"], t["ident"]
    epscol = t["epscol"]
    out_blk, dbg_outs = t["out_blk"], t["dbg_outs"]

    # ---- persistent consts ----
    cp = tc.alloc_tile_pool(name="consts", bufs=1)
    c_mask = cp.tile([128, 128], F32); nc.gpsimd.dma_start(c_mask[:], maskc[:])
    c_maskt = cp.tile([128, 128], F32); nc.gpsimd.dma_start(c_maskt[:], maskt[:])
    c_ones = cp.tile([1, 128], F32); nc.gpsimd.dma_start(c_ones[:], ones1[:])
    c_id = cp.tile([128, 128], F32); nc.gpsimd.dma_start(c_id[:], ident[:])
    c_gcol = cp.tile([128, 1], F32); nc.gpsimd.dma_start(c_gcol[:], gcol[:])
    c_b2 = cp.tile([128, E], F32); nc.gpsimd.dma_start(c_b2[:], b2bc[:])
    c_beta = cp.tile([128, E], F32); nc.gpsimd.dma_start(c_beta[:], beta_bc[:])
    c_eps = cp.tile([128, 1], F32); nc.gpsimd.dma_start(c_eps[:], epscol[:])

    # phase-scoped persistent pools (released manually, LIFO per side)
    p_att = tc.alloc_tile_pool(name="p_att", bufs=1)
    QT = [p_att.tile([128, N], F32R, tag=f"qt{h}", name=f"QT{h}") for h in range(2)]
    KT = [p_att.tile([128, N], F32R, tag=f"kt{h}", name=f"KTt{h}") for h in range(2)]
    VT = [p_att.tile([128, N], BF16, tag=f"vt{h}", name=f"VTt{h}") for h in range(2)]
    Vn = [p_att.tile([128, 16, 128], BF16, tag=f"v{h}", name=f"Vn{h}") for h in range(2)]
    HT = [p_att.tile([128, N], BF16, tag=f"ht{h}", name=f"HTt{h}") for h in range(2)]

    # ---- phase 1: QKV projections (streams xT) ----
    with tc.tile_pool(name="qkvw", bufs=1) as wp, \
         tc.tile_pool(name="xts", bufs=2) as xp, \
         tc.tile_pool(name="qkv_ps", bufs=1, space="PSUM") as qps:
        w_sb = {}
        for nm, src in (("q", wq), ("k", wk), ("v", wv)):
            w_sb[nm] = wp.tile([128, 16, 2 * D], F32R, tag="w" + nm, name="wsb_" + nm)
            nc.gpsimd.dma_start(w_sb[nm][:], src[:].rearrange("(et ep) d -> ep et d", ep=128))
        for nch in range(4):
            xt_sb = xp.tile([128, 16, 512], F32R, tag="xt")
            nc.gpsimd.dma_start(
                xt_sb[:], xT[:, nch * 512:(nch + 1) * 512].rearrange("(et ep) n -> ep et n", ep=128))
            ps = {}
            for nm in ("q", "k", "v"):
                for h in range(2):
                    ps[nm, h] = qps.tile([128, 512], F32, tag=f"ps{nm}{h}", name=f"ps_{nm}{h}")
            for et in range(16):
                for nm in ("q", "k", "v"):
                    for h in range(2):
                        nc.tensor.matmul(
                            ps[nm, h][:], w_sb[nm][:, et, h * 128:(h + 1) * 128],
                            xt_sb[:, et, :], start=(et == 0), stop=(et == 15))
            for h in range(2):
                sl = slice(nch * 512, (nch + 1) * 512)
                nc.vector.tensor_copy(QT[h][:, sl], ps["q", h][:])
                nc.vector.tensor_copy(KT[h][:, sl], ps["k", h][:])
                nc.vector.tensor_copy(VT[h][:, sl], ps["v", h][:])

    for h in range(2):
        nc.sync.dma_start_transpose(Vn[h][:], VT[h][:])

    if "d_qt" in dbg_outs: nc.gpsimd.dma_start(dbg_outs["d_qt"][:], QT[0][:])
    if "d_kt" in dbg_outs: nc.gpsimd.dma_start(dbg_outs["d_kt"][:], KT[0][:])
    if "d_vt" in dbg_outs: nc.gpsimd.dma_start(dbg_outs["d_vt"][:], VT[0][:])
    if "d_v" in dbg_outs: nc.gpsimd.dma_start(dbg_outs["d_v"][:].rearrange("p (a b) -> p a b", a=16), Vn[0][:])

    # ---- phase 2: attention per head ----
    with tc.tile_pool(name="att_sb", bufs=1) as asb, \
         tc.tile_pool(name="att_w", bufs=3) as awp, \
         tc.tile_pool(name="snat_ps", bufs=4, space="PSUM") as sps, \
         tc.tile_pool(name="stav_ps", bufs=2, space="PSUM") as tps:
        for h in range(2):
            maxcol = asb.tile([128, 16], F32, tag=f"maxcol{h}")
            sumcol = asb.tile([128, 16], F32, tag=f"sumcol{h}")
            bias1 = asb.tile([128, 16], F32, tag=f"bias1{h}")
            # --- stats via natural-layout scores ---
            for nb in range(16):
                nmch = nb // 4 + 1
                chunks = []
                for mch in range(nmch):
                    sn = sps.tile([128, 512], F32, tag="snat")
                    nc.tensor.matmul(sn[:], QT[h][:, nb * 128:(nb + 1) * 128],
                                     KT[h][:, mch * 512:(mch + 1) * 512],
                                     start=True, stop=True)
                    chunks.append(sn)
                dg = nb % 4
                nc.vector.tensor_tensor(chunks[-1][:, dg * 128:(dg + 1) * 128],
                                        chunks[-1][:, dg * 128:(dg + 1) * 128],
                                        c_mask[:], op=AluOpType.add)
                mx = awp.tile([128, 4], F32, tag="mx")
                sm = awp.tile([128, 4], F32, tag="sm")
                for mch in range(nmch):
                    w = 512 if mch < nmch - 1 else dg * 128 + 128
                    nc.vector.reduce_max(mx[:, mch:mch + 1], chunks[mch][:, 0:w],
                                         axis=mybir.AxisListType.X)
                nc.vector.reduce_max(maxcol[:, nb:nb + 1], mx[:, 0:nmch],
                                     axis=mybir.AxisListType.X)
                nc.vector.tensor_scalar_mul(bias1[:, nb:nb + 1], maxcol[:, nb:nb + 1],
                                            -float(SCALE))
                esc = awp.tile([128, 512], BF16, tag="esc")
                for mch in range(nmch):
                    w = 512 if mch < nmch - 1 else dg * 128 + 128
                    nc.scalar.activation(esc[:, 0:w], chunks[mch][:, 0:w], AF.Exp,
                                         bias=bias1[:, nb:nb + 1], scale=float(SCALE),
                                         accum_out=sm[:, mch:mch + 1])
                nc.vector.reduce_sum(sumcol[:, nb:nb + 1], sm[:, 0:nmch],
                                     axis=mybir.AxisListType.X)
            # negstat = -(lnsum)/s - max   (raw-score units)
            lncol = asb.tile([128, 16], F32, tag=f"lncol{h}")
            nc.scalar.activation(lncol[:], sumcol[:], AF.Ln)
            negstat = asb.tile([128, 16], F32, tag=f"negstat{h}")
            nc.vector.scalar_tensor_tensor(negstat[:], lncol[:], -1.0 / float(SCALE),
                                           maxcol[:], op0=AluOpType.mult,
                                           op1=AluOpType.subtract)
            stat_ps = tps.tile([16, 128], F32, tag="av")
            nc.tensor.transpose(stat_ps[:], negstat[:], c_id[:])
            statT = asb.tile([16, 128], F32, tag=f"statT{h}")
            nc.vector.tensor_copy(statT[:], stat_ps[:])
            negrow = asb.tile([1, N], F32, tag=f"negrow{h}")
            nc.gpsimd.dma_start(negrow[:].rearrange("o (a b) -> o a b", a=16), statT[:])
            if h == 0 and "d_stat" in dbg_outs:
                nc.gpsimd.dma_start(dbg_outs["d_stat"][:], negrow[:])
            # --- S^T chunks -> exp -> AV ---
            for nch in range(4):
                av = tps.tile([128, 512], F32, tag="av")
                ntile = 4 * nch + 4
                for mt in range(ntile):
                    st = tps.tile([128, 512], F32, tag="st")
                    nc.tensor.matmul(st[:], KT[h][:, mt * 128:(mt + 1) * 128],
                                     QT[h][:, nch * 512:(nch + 1) * 512],
                                     start=True, stop=False, skip_group_check=True)
                    nc.tensor.matmul(st[:], c_ones[:],
                                     negrow[:, nch * 512:(nch + 1) * 512],
                                     start=False, stop=True, skip_group_check=True)
                    off = mt * 128 - nch * 512
                    if off > 0:
                        nc.vector.memset(st[:, 0:off], NEG)
                    if off >= 0:
                        nc.vector.tensor_tensor(st[:, off:off + 128],
                                                st[:, off:off + 128],
                                                c_maskt[:], op=AluOpType.add)
                    pt = awp.tile([128, 512], BF16, tag="pt")
                    nc.scalar.activation(pt[:], st[:], AF.Exp, bias=0.0,
                                         scale=float(SCALE))
                    nc.tensor.matmul(av[:], Vn[h][:, mt, :], pt[:],
                                     start=(mt == 0), stop=(mt == ntile - 1),
                                     skip_group_check=True)
                nc.vector.tensor_copy(HT[h][:, nch * 512:(nch + 1) * 512], av[:])
    if "d_heads" in dbg_outs: nc.gpsimd.dma_start(dbg_outs["d_heads"][:], HT[0][:])

    # ---- phase 3: A2A ----
    p_pool = tc.alloc_tile_pool(name="p_pool", bufs=1, side="right")
    plhs = p_pool.tile([128, 16 * TOKPC], BF16, tag="plhs")
    with tc.tile_pool(name="dramp", bufs=1, space="DRAM") as dp:
        a2a_in = dp.tile([N, TOKPC], BF16, tag="a2ain")
        a2a_out = dp.tile([N, TOKPC], BF16, tag="a2aout")
        for h in range(2):
            nc.gpsimd.dma_start(
                a2a_in[:].rearrange("(j q d) t -> q d j t", q=2, d=128)[h],
                HT[h][:].rearrange("p (j t) -> p j t", j=8))
        nc.gpsimd.collective_compute(
            "AllToAll", AluOpType.bypass,
            ins=[a2a_in.opt()], outs=[a2a_out.opt()],
            replica_groups=[list(range(NCORE))])
        nc.gpsimd.dma_start(plhs[:].rearrange("p (k t) -> p k t", k=16),
                            a2a_out[:].rearrange("(k d) t -> d k t", d=128))
    p_att.release()

    # ---- phase 4: pool + residual + LN1 ----
    p_main = tc.alloc_tile_pool(name="p_main", bufs=1)
    z = p_pool.tile([128, 2 * E], F32, tag="z")
    y = p_main.tile([128, 2 * E], F32, tag="y")
    ybf = p_main.tile([128, 2 * E], BF16, tag="ybf")
    xb = p_pool.tile([128, 2 * E], F32, tag="xb")
    nc.gpsimd.dma_start(xb[:].rearrange("p (nb e) -> p nb e", nb=2),
                        xblk[:].rearrange("(nb p) e -> p nb e", p=128))
    with tc.tile_pool(name="wps", bufs=20) as wpp, \
         tc.tile_pool(name="pool_ps", bufs=4, space="PSUM") as pps:
        for ech in range(4):
            wts = []
            for k in range(16):
                wt = wpp.tile([128, 512], BF16, tag="wp")
                nc.gpsimd.dma_start(wt[:], wpool[k * 128:(k + 1) * 128,
                                               ech * 512:(ech + 1) * 512])
                wts.append(wt)
            for nb in range(2):
                pp = pps.tile([128, 512], F32, tag="pool")
                for k in range(16):
                    nc.tensor.matmul(pp[:], plhs[:, k * TOKPC + nb * 128:
                                                 k * TOKPC + (nb + 1) * 128],
                                     wts[k][:], start=(k == 0), stop=(k == 15))
                sl = slice(nb * E + ech * 512, nb * E + (ech + 1) * 512)
                nc.vector.tensor_tensor(z[:, sl], pp[:], xb[:, sl], op=AluOpType.add)
    if "d_pool" in dbg_outs:
        nc.gpsimd.dma_start(dbg_outs["d_pool"][:].rearrange("(nb p) e -> p nb e", p=128),
                            z[:].rearrange("p (nb e) -> p nb e", nb=2))
    _layernorm(nc, tc, z, y, c_gcol, c_beta, c_eps)
    nc.vector.tensor_copy(ybf[:], y[:])
    if "d_y" in dbg_outs:
        nc.gpsimd.dma_start(dbg_outs["d_y"][:].rearrange("(nb p) e -> p nb e", p=128),
                            y[:].rearrange("p (nb e) -> p nb e", nb=2))

    # ---- phase 5: FFN1 ----
    p_pool.release()
    yT = p_main.tile([128, 16, TOKPC], BF16, tag="yT")
    for nb in range(2):
        nc.sync.dma_start_transpose(yT[:, :, nb * 128:(nb + 1) * 128],
                                    ybf[:, nb * E:(nb + 1) * E])
    hnat = p_main.tile([128, 2, F], BF16, tag="hnat")
    with tc.tile_pool(name="w1p", bufs=8) as w1p, \
         tc.tile_pool(name="c_b1", bufs=1) as cb1, \
         tc.tile_pool(name="f1tmp", bufs=2) as f1t, \
         tc.tile_pool(name="f1_ps", bufs=4, space="PSUM") as f1ps:
        c_b1 = cb1.tile([128, F], BF16); nc.gpsimd.dma_start(c_b1[:], b1bc[:])
        for fch in range(16):
            ps2 = [f1ps.tile([128, 512], F32, tag="f1", name=f"f1ps{_}") for _ in range(2)]
            for et in range(16):
                wt = w1p.tile([128, 512], BF16, tag="w1")
                nc.gpsimd.dma_start(wt[:], w1[et * 128:(et + 1) * 128,
                                             fch * 512:(fch + 1) * 512])
                for nb in range(2):
                    nc.tensor.matmul(ps2[nb][:], yT[:, et, nb * 128:(nb + 1) * 128],
                                     wt[:], start=(et == 0), stop=(et == 15))
            for nb in range(2):
                tmp = f1t.tile([128, 512], F32, tag="f1tmp")
                nc.vector.tensor_tensor(tmp[:], ps2[nb][:],
                                        c_b1[:, fch * 512:(fch + 1) * 512],
                                        op=AluOpType.add)
                nc.vector.tensor_scalar_max(hnat[:, nb, fch * 512:(fch + 1) * 512],
                                            tmp[:], 0.0)
    if "d_h" in dbg_outs:
        nc.gpsimd.dma_start(dbg_outs["d_h"][:].rearrange("(nb p) f -> p nb f", p=128), hnat[:])

    # ---- phase 6: FFN2 + residual + LN2 ----
    hT = p_main.tile([128, 64, TOKPC], BF16, tag="hT")
    for nb in range(2):
        nc.sync.dma_start_transpose(hT[:, :, nb * 128:(nb + 1) * 128], hnat[:, nb, :])
    z2 = p_main.tile([128, 2 * E], F32, tag="z2")
    with tc.tile_pool(name="w2p", bufs=8) as w2p, \
         tc.tile_pool(name="f2_ps", bufs=4, space="PSUM") as f2ps:
        for ech in range(4):
            ps2 = [f2ps.tile([128, 512], F32, tag="f2", name=f"f2ps{_}") for _ in range(2)]
            for ft in range(64):
                wt = w2p.tile([128, 512], BF16, tag="w2")
                nc.gpsimd.dma_start(wt[:], w2[ft * 128:(ft + 1) * 128,
                                             ech * 512:(ech + 1) * 512])
                for nb in range(2):
                    nc.tensor.matmul(ps2[nb][:], hT[:, ft, nb * 128:(nb + 1) * 128],
                                     wt[:], start=(ft == 0), stop=(ft == 63))
            for nb in range(2):
                sl = slice(nb * E + ech * 512, nb * E + (ech + 1) * 512)
                nc.vector.tensor_tensor(z2[:, sl], ps2[nb][:], y[:, sl],
                                        op=AluOpType.add)
                nc.vector.tensor_tensor(z2[:, sl], z2[:, sl],
                                        c_b2[:, ech * 512:(ech + 1) * 512],
                                        op=AluOpType.add)
    out_t = p_main.tile([128, 2 * E], F32, tag="out")
    _layernorm(nc, tc, z2, out_t, c_gcol, c_beta, c_eps)
    nc.gpsimd.dma_start(out_blk[:].rearrange("(nb p) e -> p nb e", p=128),
                        out_t[:].rearrange("p (nb e) -> p nb e", nb=2))
    p_main.release()
    cp.release()


def _layernorm(nc, tc, z, out, gcol, beta, epsc):
    with tc.tile_pool(name="lnp", bufs=2) as lp:
        for nb in range(2):
            stats = lp.tile([128, 4, 6], F32, tag="bnst")
            for ch in range(4):
                nc.vector.bn_stats(stats[:, ch, :],
                                   z[:, nb * E + ch * 512: nb * E + (ch + 1) * 512])
            mv = lp.tile([128, 2], F32, tag="bnag")
            nc.vector.bn_aggr(mv[:], stats[:])
            std = lp.tile([128, 1], F32, tag="std")
            nc.scalar.activation(std[:], mv[:, 1:2], AF.Sqrt, bias=epsc[:])
            rstd = lp.tile([128, 1], F32, tag="rstd")
            nc.vector.reciprocal(rstd[:], std[:])
            rg = lp.tile([128, 1], F32, tag="rg")
            nc.vector.tensor_tensor(rg[:], rstd[:], gcol[:], op=AluOpType.mult)
            sl = slice(nb * E, (nb + 1) * E)
            nc.vector.tensor_scalar(out[:, sl], z[:, sl], mv[:, 0:1], rg[:],
                                    AluOpType.subtract, AluOpType.mult)
            nc.vector.tensor_tensor(out[:, sl], out[:, sl], beta[:], op=AluOpType.add)


def round11(a):
    u = np.ascontiguousarray(a, dtype=np.float32).view(np.uint32).astype(np.uint64)
    return ((u + np.uint64(0x800)) & np.uint64(0xFFFFF000)).astype(np.uint32).view(np.float32)


def prep_inputs(inp):
    """Full reference inputs -> list of 8 per-core input dicts."""
    x = np.asarray(inp["token_embeddings"], np.float32)
    WQ = np.asarray(inp["W_Q"], np.float32); WK = np.asarray(inp["W_K"], np.float32)
    WV = np.asarray(inp["W_V"], np.float32); WP = np.asarray(inp["W_Pool"], np.float32)
    W1 = np.asarray(inp["W_1"], np.float32); b1 = np.asarray(inp["b_1"], np.float32)
    W2 = np.asarray(inp["W_2"], np.float32); b2 = np.asarray(inp["b_2"], np.float32)
    gamma = np.asarray(inp["gamma"], np.float32); beta = np.asarray(inp["beta"], np.float32)
    bf = ml_dtypes.bfloat16
    xT = round11(np.ascontiguousarray(x.T))
    shared = {
        "xT": xT,
        "wpool": WP.astype(bf),
        "w1": W1.astype(bf),
        "w2": W2.astype(bf),
        "b1bc": np.broadcast_to(b1.reshape(1, F), (128, F)).astype(bf).copy(),
        "b2bc": np.broadcast_to(b2.reshape(1, E), (128, E)).astype(np.float32).copy(),
        "beta_bc": np.broadcast_to(beta.reshape(1, E), (128, E)).astype(np.float32).copy(),
        "gcol": np.full((128, 1), float(gamma.reshape(-1)[0]), np.float32),
        "maskc": np.where(np.arange(128)[None, :] <= np.arange(128)[:, None], 0.0, NEG).astype(np.float32),
        "maskt": np.where(np.arange(128)[:, None] <= np.arange(128)[None, :], 0.0, NEG).astype(np.float32),
        "ones1": np.ones((1, 128), np.float32),
        "epscol": np.full((128, 1), EPS, np.float32),
        "ident": np.eye(128, dtype=np.float32),
    }
    maps = []
    for c in range(NCORE):
        m = dict(shared)
        m["wq"] = round11(np.concatenate([WQ[2 * c], WQ[2 * c + 1]], axis=1))
        m["wk"] = round11(np.concatenate([WK[2 * c], WK[2 * c + 1]], axis=1))
        m["wv"] = round11(np.concatenate([WV[2 * c], WV[2 * c + 1]], axis=1))
        m["xblk"] = np.ascontiguousarray(x[c * TOKPC:(c + 1) * TOKPC])
        maps.append(m)
    return maps


def assemble(results):
    return np.concatenate([r["out_blk"] for r in results], axis=0)


# ----------------------------------------------------------------------------
# PJRT execution (axon): jit once, reuse.
# ----------------------------------------------------------------------------
import jax
from concourse.bass2jax import _bass_exec_p, install_neuronx_cc_hook, partition_id_tensor
from jax.sharding import Mesh, PartitionSpec
from jax.experimental.shard_map import shard_map


class _Runner:
    def __init__(self, nc, n_cores):
        install_neuronx_cc_hook()
        self.nc = nc
        self.n_cores = n_cores
        in_names, out_names, out_avals, zero_outs = [], [], [], []
        for alloc in nc.m.functions[0].allocations:
            if not isinstance(alloc, mybir.MemoryLocationSet):
                continue
            name = alloc.memorylocations[0].name
            if alloc.kind == "ExternalInput":
                in_names.append(name)
            elif alloc.kind == "ExternalOutput":
                out_names.append(name)
                shape = tuple(alloc.tensor_shape)
                dtype = mybir.dt.np(alloc.dtype)
                out_avals.append(jax.core.ShapedArray(shape, dtype))
                zero_outs.append(np.zeros(shape, dtype))
        self.partition_name = nc.partition_id_tensor.name if nc.partition_id_tensor else None
        if self.partition_name in in_names:
            in_names.remove(self.partition_name)
        self.in_names = list(in_names)
        self.out_names = out_names
        self.out_avals = out_avals
        self.zero_outs = zero_outs
        self.n_params = len(in_names)
        all_in_names = in_names + out_names
        if self.partition_name is not None:
            all_in_names.append(self.partition_name)
        partition_name = self.partition_name

        def _body(*args):
            operands = list(args)
            if partition_name is not None:
                operands.append(partition_id_tensor())
            outs = _bass_exec_p.bind(
                *operands,
                out_avals=tuple(out_avals),
                in_names=tuple(all_in_names),
                out_names=tuple(out_names),
                lowering_input_output_aliases=(),
                sim_require_finite=True,
                sim_require_nnan=True,
                nc=nc,
            )
            return tuple(outs)

        devices = jax.devices()[:n_cores]
        self.mesh = Mesh(np.asarray(devices), ("core",))
        n_outs = len(out_avals)
        in_specs = (PartitionSpec("core"),) * (self.n_params + n_outs)
        out_specs = (PartitionSpec("core"),) * len(out_names)
        self.fn = jax.jit(
            shard_map(_body, mesh=self.mesh, in_specs=in_specs,
                      out_specs=out_specs, check_rep=False),
            keep_unused=True)

    def prep(self, in_maps):
        per_core = [[np.asarray(m[n]) for n in self.in_names] for m in in_maps]
        concat_in = [np.concatenate([per_core[c][i] for c in range(self.n_cores)], axis=0)
                     for i in range(self.n_params)]
        concat_zeros = [np.zeros((self.n_cores * z.shape[0], *z.shape[1:]), z.dtype)
                        for z in self.zero_outs]
        sh = jax.sharding.NamedSharding(self.mesh, PartitionSpec("core"))
        return [jax.device_put(a, sh) for a in concat_in + concat_zeros]

    def run(self, args):
        outs = self.fn(*args)
        jax.block_until_ready(outs)
        return outs

    def results(self, outs):
        return [
            {n: np.asarray(outs[i]).reshape(self.n_cores, *self.out_avals[i].shape)[c]
             for i, n in enumerate(self.out_names)}
            for c in range(self.n_cores)
        ]


_CACHE = {}


def _get_runner():
    if "r" not in _CACHE:
        nc, _ = build_nc()
        _CACHE["r"] = _Runner(nc, NCORE)
    return _CACHE["r"]


def kernel(**inputs):
    r = _get_runner()
    maps = prep_inputs(inputs)
    args = r.prep(maps)
    outs = r.run(args)
    return assemble(r.results(outs)).astype(np.float32)

